# revision 8
# baseline (speedup 1.0000x reference)
"""2D DCT-II (ortho) over the last two axes of x[8, 32, 512, 512] (f32),
data-parallel across 8 NeuronCores (one batch element per core).

Per core, for each of 32 images X (512x512): Y = D @ X @ D^T.
matmul(out, lhsT, rhs) = lhsT.T @ rhs, so chaining two matmuls with
lhsT = data gives D X D^T with no explicit transposes:
  stage 1: Z = matmul(lhsT=X*, rhs=DT) = (D X*)^T
  stage 2: Y = matmul(lhsT=Z,  rhs=..)

Stage 2 is halved with the even/odd DCT split: fold X along its free
dim (Xe/Xo = X[:, i] +/- X[:, 511-i]) before stage 1; then
Y[:, 2k] comes from Ze against A = D[0::2, :256] and Y[:, 2k+1] from
Zo against B = D[1::2, :256], each a 256-contraction.

All matmul operands are bf16: f32r weight loads run at 4B/col (~192ns
per 128-col LDWEIGHTS) and made the baseline ldweights-bound; bf16
loads (~107ns) overlap under the matmuls, dropping tensor time below
the DMA roofline. DCT matrices are pre-rounded to bf16 on host; the
column fold casts x to bf16 on the fly.

DMA layouts put 4 consecutive image rows on one partition
("(p ro) c") so each partition line is one 8KB contiguous HBM
segment (vs 4x2KB with the "(ro p)" layout) for both the x load and
the y store; stage-2 takes its lhsT as a stride-4 slice of z's free
dim so output rows land as u = 4p+uo.
"""
import numpy as np
import ml_dtypes

import concourse.bass as bass
import concourse.mybir as mybir
import concourse.tile as tile
from concourse.bass_utils import run_bass_kernel_spmd

P = 128
N = 512
H = N // 2          # 256
KO = N // P         # 4
HO = H // P         # 2
NIMG = 32
NCORES = 8

_MAX_WAITS = 1


def _split_excess_waits(nc):
    """walrus CoreV3 codegen rejects instructions carrying several sem
    waits; hoist excess waits onto preceding same-engine NoOps."""
    for f in nc.m.functions:
        for bb in f.blocks:
            insts = bb.instructions
            i = 0
            while i < len(insts):
                inst = insts[i]
                si = inst.sync_info
                if si is not None and si.on_wait and len(si.on_wait) > _MAX_WAITS:
                    waits = list(si.on_wait)
                    keep = waits[-_MAX_WAITS:]
                    hoist = waits[:-_MAX_WAITS]
                    nops = []
                    for w in hoist:
                        nop = mybir.InstNoOp(
                            name=nc.get_next_instruction_name(), ins=[], outs=[])
                        nop.engine = inst.engine
                        nop.sync_info = mybir.SyncInfo(on_wait=[w], on_update=[])
                        nops.append(nop)
                    si.on_wait = keep
                    for off, nop in enumerate(nops):
                        insts.insert(i + off, nop)
                    i += len(nops)
                i += 1


def _dct_mats():
    k = np.arange(N)[:, None]
    j = np.arange(N)[None, :]
    D = np.cos(np.pi * (2 * j + 1) * k / (2.0 * N))
    D *= np.sqrt(2.0 / N)
    D[0] *= 1.0 / np.sqrt(2.0)
    D = D.astype(np.float64)
    # row orders match the SBUF layouts: DT rows r laid out r = 4p+ro,
    # AT/BT rows i laid out i = ic*128+p.
    DT = D.T.astype(ml_dtypes.bfloat16)              # [r, u]
    AT = D[0::2, :H].T.astype(ml_dtypes.bfloat16)    # [i, k] even rows
    BT = D[1::2, :H].T.astype(ml_dtypes.bfloat16)    # [i, k] odd rows
    return (np.ascontiguousarray(DT), np.ascontiguousarray(AT),
            np.ascontiguousarray(BT))


def _build():
    nc = bass.Bass()
    f32 = mybir.dt.float32
    bf16 = mybir.dt.bfloat16
    x_d = nc.dram_tensor("x", [NIMG, N, N], f32, kind="ExternalInput")
    dt_d = nc.dram_tensor("dt", [N, N], bf16, kind="ExternalInput")
    at_d = nc.dram_tensor("at", [H, H], bf16, kind="ExternalInput")
    bt_d = nc.dram_tensor("bt", [H, H], bf16, kind="ExternalInput")
    y_d = nc.dram_tensor("y", [NIMG, N, N], f32, kind="ExternalOutput")

    with tile.TileContext(nc) as tc:
        with (
            tc.tile_pool(name="const", bufs=1) as cpool,
            tc.tile_pool(name="xp", bufs=8) as xp,
            tc.tile_pool(name="fp", bufs=3) as fp,
            tc.tile_pool(name="zp", bufs=4) as zp,
            tc.tile_pool(name="yp", bufs=5) as yp,
            tc.tile_pool(name="ps", bufs=4, space="PSUM") as ps1p,
            tc.tile_pool(name="ps2", bufs=4, space="PSUM") as ps2p,
        ):
            # dt_mm[p, ro, u] = DT[4p+ro, u]
            dt_mm = cpool.tile([P, KO, N], bf16, tag="dt")
            nc.sync.dma_start(dt_mm[:], dt_d.rearrange("(p ro) u -> p ro u", p=P))
            # at_mm[p, ic, k] = AT[ic*128+p, k]
            ab_mm = cpool.tile([P, 2 * HO, H], bf16, tag="ab")
            nc.sync.dma_start(
                ab_mm[:, 0:HO, :], at_d.rearrange("(ic p) k -> p ic k", p=P))
            nc.sync.dma_start(
                ab_mm[:, HO:2 * HO, :], bt_d.rearrange("(ic p) k -> p ic k", p=P))
            at_mm = ab_mm[:, 0:HO, :]
            bt_mm = ab_mm[:, HO:2 * HO, :]

            for img in range(NIMG):
                # x_sb[p, ro, c] = x[img, 4p+ro, c]: one 8KB contiguous
                # HBM segment per partition.
                x_sb = xp.tile([P, KO, N], f32)
                nc.sync.dma_start(
                    x_sb[:].rearrange("p ro c -> p (ro c)"),
                    x_d[img].rearrange("(p ro) c -> p (ro c)", p=P))

                # free-dim fold -> bf16 (the fold op does the rounding)
                xf = fp.tile([P, 2, KO, H], bf16, tag="xf")
                xe = xf[:, 0]
                xo = xf[:, 1]
                xrev = x_sb[:, :, N - 1:H - 1:-1]
                nc.vector.tensor_add(xe, x_sb[:, :, 0:H], xrev)
                nc.vector.tensor_sub(xo, x_sb[:, :, 0:H], xrev)

                # stage 1: Ze/Zo = (D Xe/Xo)^T, z[p, part*HO+ic, u],
                # partition p holds folded-column index i = ic*128+p.
                z_sb = zp.tile([P, 2 * HO, N], bf16)
                for part in range(2):
                    src = xf[:, part]
                    for ic in range(HO):
                        pz = ps1p.tile([P, N], f32, tag="ps1")
                        for ro in range(KO):
                            nc.tensor.matmul(
                                pz[:],
                                src[:, ro, ic * P:(ic + 1) * P],
                                dt_mm[:, ro, :],
                                start=(ro == 0),
                                stop=(ro == KO - 1),
                            )
                        nc.vector.tensor_copy(z_sb[:, part * HO + ic, :], pz[:])
                ze = z_sb[:, 0:HO, :]
                zo = z_sb[:, HO:2 * HO, :]

                # stage 2: matmuls write PSUM with stride-2 free APs so
                # even/odd columns interleave in place and the output
                # copy is contiguous; lhsT is a stride-4 slice of z's
                # free dim so py partition p holds output row u = 4p+uo.
                y_sb = yp.tile([P, KO, N], f32)
                for uo in range(KO):
                    py = ps2p.tile([P, N], f32, tag="ps2")
                    for ic in range(HO):
                        nc.tensor.matmul(
                            py[:, 0:N:2],
                            ze[:, ic, uo:uo + 4 * (P - 1) + 1:4],
                            at_mm[:, ic, :],
                            start=(ic == 0),
                            stop=(ic == HO - 1),
                        )
                    for ic in range(HO):
                        nc.tensor.matmul(
                            py[:, 1:N:2],
                            zo[:, ic, uo:uo + 4 * (P - 1) + 1:4],
                            bt_mm[:, ic, :],
                            start=(ic == 0),
                            stop=(ic == HO - 1),
                        )
                    nc.scalar.copy(y_sb[:, uo, :], py[:])
                # y-out rides the scalar HWDGE ring so input prefetch on
                # the sync ring never queues behind output readiness.
                nc.scalar.dma_start(
                    y_d[img].rearrange("(p uo) v -> p (uo v)", p=P),
                    y_sb[:].rearrange("p uo v -> p (uo v)"))

    _split_excess_waits(nc)
    return nc


_CACHE = {}


def _get_nc():
    if "nc" not in _CACHE:
        _CACHE["nc"] = _build()
    return _CACHE["nc"]


def _in_maps(x):
    dt, at, bt = _dct_mats()
    return [{"x": x[i], "dt": dt, "at": at, "bt": bt} for i in range(NCORES)]


def kernel(x):
    x = np.ascontiguousarray(np.asarray(x, dtype=np.float32))
    assert x.shape == (NCORES, NIMG, N, N), x.shape
    nc = _get_nc()
    res = run_bass_kernel_spmd(nc, _in_maps(x), core_ids=list(range(NCORES)))
    out = np.stack([res.results[i]["y"] for i in range(NCORES)], axis=0)
    return out.astype(np.float32)


# revision 10
# speedup vs baseline: 1.0712x; 1.0712x over previous
"""2D DCT-II (ortho) over the last two axes of x[8, 32, 512, 512] (f32),
data-parallel across 8 NeuronCores (one batch element per core).

Per core, for each of 32 images X (512x512): Y = D @ X @ D^T.
matmul(out, lhsT, rhs) = lhsT.T @ rhs, so chaining two matmuls with
lhsT = data gives D X D^T with no explicit transposes:
  stage 1: Z = matmul(lhsT=X*, rhs=DT) = (D X*)^T
  stage 2: Y = matmul(lhsT=Z,  rhs=..)

Stage 2 is halved with the even/odd DCT split: fold X along its free
dim (Xe/Xo = X[:, i] +/- X[:, 511-i]) before stage 1; then
Y[:, 2k] comes from Ze against A = D[0::2, :256] and Y[:, 2k+1] from
Zo against B = D[1::2, :256], each a 256-contraction.

All matmul operands are bf16: f32r weight loads run at 4B/col (~192ns
per 128-col LDWEIGHTS) and made the baseline ldweights-bound; bf16
loads (~107ns) overlap under the matmuls, dropping tensor time below
the DMA roofline. DCT matrices are pre-rounded to bf16 on host; the
column fold casts x to bf16 on the fly.

DMA layouts put 4 consecutive image rows on one partition
("(p ro) c") so each partition line is one 8KB contiguous HBM
segment (vs 4x2KB with the "(ro p)" layout) for both the x load and
the y store; stage-2 takes its lhsT as a stride-4 slice of z's free
dim so output rows land as u = 4p+uo.
"""
import numpy as np
import ml_dtypes

import concourse.bass as bass
import concourse.mybir as mybir
import concourse.tile as tile
from concourse.bass_utils import run_bass_kernel_spmd

P = 128
N = 512
H = N // 2          # 256
KO = N // P         # 4
HO = H // P         # 2
NIMG = 32
NCORES = 8

_MAX_WAITS = 1


def _split_excess_waits(nc):
    """walrus CoreV3 codegen rejects instructions carrying several sem
    waits; hoist excess waits onto preceding same-engine NoOps."""
    for f in nc.m.functions:
        for bb in f.blocks:
            insts = bb.instructions
            i = 0
            while i < len(insts):
                inst = insts[i]
                si = inst.sync_info
                if si is not None and si.on_wait and len(si.on_wait) > _MAX_WAITS:
                    waits = list(si.on_wait)
                    keep = waits[-_MAX_WAITS:]
                    hoist = waits[:-_MAX_WAITS]
                    nops = []
                    for w in hoist:
                        nop = mybir.InstNoOp(
                            name=nc.get_next_instruction_name(), ins=[], outs=[])
                        nop.engine = inst.engine
                        nop.sync_info = mybir.SyncInfo(on_wait=[w], on_update=[])
                        nops.append(nop)
                    si.on_wait = keep
                    for off, nop in enumerate(nops):
                        insts.insert(i + off, nop)
                    i += len(nops)
                i += 1


def _dct_mats():
    k = np.arange(N)[:, None]
    j = np.arange(N)[None, :]
    D = np.cos(np.pi * (2 * j + 1) * k / (2.0 * N))
    D *= np.sqrt(2.0 / N)
    D[0] *= 1.0 / np.sqrt(2.0)
    D = D.astype(np.float64)
    # row orders match the SBUF layouts: DT rows r laid out r = 4p+ro,
    # AT/BT rows i laid out i = ic*128+p.
    DT = D.T.astype(ml_dtypes.bfloat16)              # [r, u]
    AT = D[0::2, :H].T.astype(ml_dtypes.bfloat16)    # [i, k] even rows
    BT = D[1::2, :H].T.astype(ml_dtypes.bfloat16)    # [i, k] odd rows
    return (np.ascontiguousarray(DT), np.ascontiguousarray(AT),
            np.ascontiguousarray(BT))


def _build():
    nc = bass.Bass()
    f32 = mybir.dt.float32
    bf16 = mybir.dt.bfloat16
    x_d = nc.dram_tensor("x", [NIMG, N, N], f32, kind="ExternalInput")
    dt_d = nc.dram_tensor("dt", [N, N], bf16, kind="ExternalInput")
    at_d = nc.dram_tensor("at", [H, H], bf16, kind="ExternalInput")
    bt_d = nc.dram_tensor("bt", [H, H], bf16, kind="ExternalInput")
    y_d = nc.dram_tensor("y", [NIMG, N, N], f32, kind="ExternalOutput")

    with tile.TileContext(nc) as tc:
        with (
            tc.tile_pool(name="const", bufs=1) as cpool,
            tc.tile_pool(name="xp", bufs=8) as xp,
            tc.tile_pool(name="fp", bufs=3) as fp,
            tc.tile_pool(name="zp", bufs=4) as zp,
            tc.tile_pool(name="yp", bufs=5) as yp,
            tc.tile_pool(name="ps", bufs=4, space="PSUM") as ps1p,
            tc.tile_pool(name="ps2", bufs=4, space="PSUM") as ps2p,
        ):
            # dt_mm[p, ro, u] = DT[4p+ro, u]
            dt_mm = cpool.tile([P, KO, N], bf16, tag="dt")
            nc.sync.dma_start(dt_mm[:], dt_d.rearrange("(p ro) u -> p ro u", p=P))
            # at_mm[p, ic, k] = AT[ic*128+p, k]
            ab_mm = cpool.tile([P, 2 * HO, H], bf16, tag="ab")
            nc.sync.dma_start(
                ab_mm[:, 0:HO, :], at_d.rearrange("(ic p) k -> p ic k", p=P))
            nc.sync.dma_start(
                ab_mm[:, HO:2 * HO, :], bt_d.rearrange("(ic p) k -> p ic k", p=P))
            at_mm = ab_mm[:, 0:HO, :]
            bt_mm = ab_mm[:, HO:2 * HO, :]

            for img in range(NIMG):
                # x_sb[p, ro, c] = x[img, 4p+ro, c]: one 8KB contiguous
                # HBM segment per partition.
                x_sb = xp.tile([P, KO, N], f32)
                nc.sync.dma_start(
                    x_sb[:].rearrange("p ro c -> p (ro c)"),
                    x_d[img].rearrange("(p ro) c -> p (ro c)", p=P))

                # free-dim fold -> bf16 (the fold op does the rounding)
                xf = fp.tile([P, 2, KO, H], bf16, tag="xf")
                xe = xf[:, 0]
                xo = xf[:, 1]
                xrev = x_sb[:, :, N - 1:H - 1:-1]
                nc.vector.tensor_add(xe, x_sb[:, :, 0:H], xrev)
                nc.vector.tensor_sub(xo, x_sb[:, :, 0:H], xrev)

                # stage 1: Ze/Zo = (D Xe/Xo)^T, z[p, part*HO+ic, u],
                # partition p holds folded-column index i = ic*128+p.
                z_sb = zp.tile([P, 2 * HO, N], bf16)
                for part in range(2):
                    src = xf[:, part]
                    for ic in range(HO):
                        pz = ps1p.tile([P, N], f32, tag="ps1")
                        for ro in range(KO):
                            nc.tensor.matmul(
                                pz[:],
                                src[:, ro, ic * P:(ic + 1) * P],
                                dt_mm[:, ro, :],
                                start=(ro == 0),
                                stop=(ro == KO - 1),
                            )
                        if part == 0:
                            nc.vector.tensor_copy(
                                z_sb[:, part * HO + ic, :], pz[:])
                        else:
                            nc.scalar.copy(z_sb[:, part * HO + ic, :], pz[:])
                ze = z_sb[:, 0:HO, :]
                zo = z_sb[:, HO:2 * HO, :]

                # stage 2: py[:, 0:H] even v, [:, H:N] odd v; lhsT is a
                # stride-4 slice of z's free dim so py partition p holds
                # output row u = 4p+uo. (Strided PSUM matmul writes would
                # fuse the interleave but halve PE writeback rate.)
                y_sb = yp.tile([P, KO, N], f32)
                for uo in range(KO):
                    py = ps2p.tile([P, N], f32, tag="ps2")
                    for ic in range(HO):
                        nc.tensor.matmul(
                            py[:, 0:H],
                            ze[:, ic, uo:uo + 4 * (P - 1) + 1:4],
                            at_mm[:, ic, :],
                            start=(ic == 0),
                            stop=(ic == HO - 1),
                        )
                    for ic in range(HO):
                        nc.tensor.matmul(
                            py[:, H:N],
                            zo[:, ic, uo:uo + 4 * (P - 1) + 1:4],
                            bt_mm[:, ic, :],
                            start=(ic == 0),
                            stop=(ic == HO - 1),
                        )
                    # interleave: y[p, uo, 2k+t] = py[p, t*H + k]
                    src_ap = py[:].rearrange("p (two k) -> p two k", two=2)
                    dst_ap = y_sb[:, uo, :].rearrange(
                        "p (k two) -> p two k", two=2)
                    if uo == 0:
                        nc.vector.tensor_copy(dst_ap, src_ap)
                    else:
                        nc.scalar.copy(dst_ap, src_ap)
                # y-out rides the scalar HWDGE ring so input prefetch on
                # the sync ring never queues behind output readiness.
                nc.scalar.dma_start(
                    y_d[img].rearrange("(p uo) v -> p (uo v)", p=P),
                    y_sb[:].rearrange("p uo v -> p (uo v)"))

    _split_excess_waits(nc)
    return nc


_CACHE = {}


def _get_nc():
    if "nc" not in _CACHE:
        _CACHE["nc"] = _build()
    return _CACHE["nc"]


def _in_maps(x):
    dt, at, bt = _dct_mats()
    return [{"x": x[i], "dt": dt, "at": at, "bt": bt} for i in range(NCORES)]


def kernel(x):
    x = np.ascontiguousarray(np.asarray(x, dtype=np.float32))
    assert x.shape == (NCORES, NIMG, N, N), x.shape
    nc = _get_nc()
    res = run_bass_kernel_spmd(nc, _in_maps(x), core_ids=list(range(NCORES)))
    out = np.stack([res.results[i]["y"] for i in range(NCORES)], axis=0)
    return out.astype(np.float32)


# revision 11
# speedup vs baseline: 1.1411x; 1.0652x over previous
"""2D DCT-II (ortho) over the last two axes of x[8, 32, 512, 512] (f32),
data-parallel across 8 NeuronCores (one batch element per core).

Per core, for each of 32 images X (512x512): Y = D @ X @ D^T.
matmul(out, lhsT, rhs) = lhsT.T @ rhs, so chaining two matmuls with
lhsT = data gives D X D^T with no explicit transposes:
  stage 1: Z = matmul(lhsT=X*, rhs=DT) = (D X*)^T
  stage 2: Y = matmul(lhsT=Z,  rhs=..)

Stage 2 is halved with the even/odd DCT split: fold X along its free
dim (Xe/Xo = X[:, i] +/- X[:, 511-i]) before stage 1; then
Y[:, 2k] comes from Ze against A = D[0::2, :256] and Y[:, 2k+1] from
Zo against B = D[1::2, :256], each a 256-contraction.

All matmul operands are bf16: f32r weight loads run at 4B/col (~192ns
per 128-col LDWEIGHTS) and made the baseline ldweights-bound; bf16
loads (~107ns) overlap under the matmuls, dropping tensor time below
the DMA roofline. DCT matrices are pre-rounded to bf16 on host; the
column fold casts x to bf16 on the fly.

DMA layouts put 4 consecutive image rows on one partition
("(p ro) c") so each partition line is one 8KB contiguous HBM
segment (vs 4x2KB with the "(ro p)" layout) for both the x load and
the y store; stage-2 takes its lhsT as a stride-4 slice of z's free
dim so output rows land as u = 4p+uo.
"""
import numpy as np
import ml_dtypes

import concourse.bass as bass
import concourse.mybir as mybir
import concourse.tile as tile
from concourse.bass_utils import run_bass_kernel_spmd

P = 128
N = 512
H = N // 2          # 256
KO = N // P         # 4
HO = H // P         # 2
NIMG = 32
NCORES = 8

_MAX_WAITS = 1


def _split_excess_waits(nc):
    """walrus CoreV3 codegen rejects instructions carrying several sem
    waits; hoist excess waits onto preceding same-engine NoOps."""
    for f in nc.m.functions:
        for bb in f.blocks:
            insts = bb.instructions
            i = 0
            while i < len(insts):
                inst = insts[i]
                si = inst.sync_info
                if si is not None and si.on_wait and len(si.on_wait) > _MAX_WAITS:
                    waits = list(si.on_wait)
                    keep = waits[-_MAX_WAITS:]
                    hoist = waits[:-_MAX_WAITS]
                    nops = []
                    for w in hoist:
                        nop = mybir.InstNoOp(
                            name=nc.get_next_instruction_name(), ins=[], outs=[])
                        nop.engine = inst.engine
                        nop.sync_info = mybir.SyncInfo(on_wait=[w], on_update=[])
                        nops.append(nop)
                    si.on_wait = keep
                    for off, nop in enumerate(nops):
                        insts.insert(i + off, nop)
                    i += len(nops)
                i += 1


def _dct_mats():
    k = np.arange(N)[:, None]
    j = np.arange(N)[None, :]
    D = np.cos(np.pi * (2 * j + 1) * k / (2.0 * N))
    D *= np.sqrt(2.0 / N)
    D[0] *= 1.0 / np.sqrt(2.0)
    D = D.astype(np.float64)
    # row orders match the SBUF layouts: DT rows r laid out r = 4p+ro,
    # AT/BT rows i laid out i = ic*128+p.
    DT = D.T.astype(ml_dtypes.bfloat16)              # [r, u]
    AT = D[0::2, :H].T.astype(ml_dtypes.bfloat16)    # [i, k] even rows
    BT = D[1::2, :H].T.astype(ml_dtypes.bfloat16)    # [i, k] odd rows
    return (np.ascontiguousarray(DT), np.ascontiguousarray(AT),
            np.ascontiguousarray(BT))


def _build():
    nc = bass.Bass()
    f32 = mybir.dt.float32
    bf16 = mybir.dt.bfloat16
    x_d = nc.dram_tensor("x", [NIMG, N, N], f32, kind="ExternalInput")
    dt_d = nc.dram_tensor("dt", [N, N], bf16, kind="ExternalInput")
    at_d = nc.dram_tensor("at", [H, H], bf16, kind="ExternalInput")
    bt_d = nc.dram_tensor("bt", [H, H], bf16, kind="ExternalInput")
    y_d = nc.dram_tensor("y", [NIMG, N, N], f32, kind="ExternalOutput")

    with tile.TileContext(nc) as tc:
        with (
            tc.tile_pool(name="const", bufs=1) as cpool,
            tc.tile_pool(name="xp", bufs=6) as xp,
            tc.tile_pool(name="fp", bufs=3) as fp,
            tc.tile_pool(name="zp", bufs=3) as zp,
            tc.tile_pool(name="yp", bufs=4) as yp,
            tc.tile_pool(name="ps", bufs=4, space="PSUM") as ps1p,
            tc.tile_pool(name="ps2", bufs=4, space="PSUM") as ps2p,
        ):
            # dt_mm[p, ro, u] = DT[4p+ro, u]
            dt_mm = cpool.tile([P, KO, N], bf16, tag="dt")
            nc.sync.dma_start(dt_mm[:], dt_d.rearrange("(p ro) u -> p ro u", p=P))
            # at_mm[p, ic, k] = AT[ic*128+p, k]
            ab_mm = cpool.tile([P, 2 * HO, H], bf16, tag="ab")
            nc.sync.dma_start(
                ab_mm[:, 0:HO, :], at_d.rearrange("(ic p) k -> p ic k", p=P))
            nc.sync.dma_start(
                ab_mm[:, HO:2 * HO, :], bt_d.rearrange("(ic p) k -> p ic k", p=P))
            at_mm = ab_mm[:, 0:HO, :]
            bt_mm = ab_mm[:, HO:2 * HO, :]

            for img in range(NIMG):
                # x_sb[p, ro, c] = x[img, 4p+ro, c]: one 8KB contiguous
                # HBM segment per partition.
                x_sb = xp.tile([P, KO, N], f32)
                nc.sync.dma_start(
                    x_sb[:].rearrange("p ro c -> p (ro c)"),
                    x_d[img].rearrange("(p ro) c -> p (ro c)", p=P))

                # free-dim fold -> bf16 (the fold op does the rounding)
                xf = fp.tile([P, 2, KO, H], bf16, tag="xf")
                xe = xf[:, 0]
                xo = xf[:, 1]
                xrev = x_sb[:, :, N - 1:H - 1:-1]
                nc.vector.tensor_add(xe, x_sb[:, :, 0:H], xrev)
                nc.vector.tensor_sub(xo, x_sb[:, :, 0:H], xrev)

                # stage 1: Ze/Zo = (D Xe/Xo)^T, z[p, part*HO+ic, u],
                # partition p holds folded-column index i = ic*128+p.
                z_sb = zp.tile([P, 2 * HO, N], bf16)
                for part in range(2):
                    src = xf[:, part]
                    for ic in range(HO):
                        pz = ps1p.tile([P, N], f32, tag="ps1")
                        for ro in range(KO):
                            nc.tensor.matmul(
                                pz[:],
                                src[:, ro, ic * P:(ic + 1) * P],
                                dt_mm[:, ro, :],
                                start=(ro == 0),
                                stop=(ro == KO - 1),
                            )
                        nc.scalar.copy(z_sb[:, part * HO + ic, :], pz[:])
                ze = z_sb[:, 0:HO, :]
                zo = z_sb[:, HO:2 * HO, :]

                # stage 2: py[:, 0:H] even v, [:, H:N] odd v; lhsT is a
                # stride-4 slice of z's free dim so py partition p holds
                # output row u = 4p+uo.
                y_sb = yp.tile([P, KO, N], f32)
                for uo in range(KO):
                    py = ps2p.tile([P, N], f32, tag="ps2")
                    for ic in range(HO):
                        nc.tensor.matmul(
                            py[:, 0:H],
                            ze[:, ic, uo:uo + 4 * (P - 1) + 1:4],
                            at_mm[:, ic, :],
                            start=(ic == 0),
                            stop=(ic == HO - 1),
                        )
                    for ic in range(HO):
                        nc.tensor.matmul(
                            py[:, H:N],
                            zo[:, ic, uo:uo + 4 * (P - 1) + 1:4],
                            bt_mm[:, ic, :],
                            start=(ic == 0),
                            stop=(ic == HO - 1),
                        )
                    # interleave: y[p, uo, 2k+t] = py[p, t*H + k]
                    src_ap = py[:].rearrange("p (two k) -> p two k", two=2)
                    dst_ap = y_sb[:, uo, :].rearrange(
                        "p (k two) -> p two k", two=2)
                    nc.scalar.copy(dst_ap, src_ap)
                nc.sync.dma_start(
                    y_d[img].rearrange("(p uo) v -> p (uo v)", p=P),
                    y_sb[:].rearrange("p uo v -> p (uo v)"))

    _split_excess_waits(nc)
    return nc


_CACHE = {}


def _get_nc():
    if "nc" not in _CACHE:
        _CACHE["nc"] = _build()
    return _CACHE["nc"]


def _in_maps(x):
    dt, at, bt = _dct_mats()
    return [{"x": x[i], "dt": dt, "at": at, "bt": bt} for i in range(NCORES)]


def kernel(x):
    x = np.ascontiguousarray(np.asarray(x, dtype=np.float32))
    assert x.shape == (NCORES, NIMG, N, N), x.shape
    nc = _get_nc()
    res = run_bass_kernel_spmd(nc, _in_maps(x), core_ids=list(range(NCORES)))
    out = np.stack([res.results[i]["y"] for i in range(NCORES)], axis=0)
    return out.astype(np.float32)
